# revision 14
# baseline (speedup 1.0000x reference)
# GATv2 5-layer GNN + multi-aggregation readout + MLP head on 8 TRN2 NeuronCores.
#
# Sharding: 256 graphs -> 32 contiguous graphs per core (batch_index-sorted).
# Edges sharded by destination node.  Per layer each core computes node
# transforms for its slice, AllGathers the source transform xl (bf16), and runs
# softmax-attention aggregation over 896-edge tiles with matmul segment sums.
#
# kernel(**inputs) -> np.ndarray [256, 10] float32

import sys

for _p in ("/opt/trn_rl_repo", "/root/.axon_site/_ro/trn_rl_repo"):
    if _p not in sys.path:
        sys.path.append(_p)

import numpy as np
import ml_dtypes

BF16 = ml_dtypes.bfloat16

N_CORES = 8
HEADS = 3
NEG_SLOPE = 0.2
BN_EPS = 1e-5
P = 128              # partitions
CHUNKS = 7           # 128-edge chunks per edge tile
TILE_E = P * CHUNKS  # 896 edges per tile
UNROLL = 4


# ----------------------------------------------------------------------------
# Host-side preprocessing
# ----------------------------------------------------------------------------

def _build_edge_tiles(src, dst, lo, hi, np_pad, rank_of_node, local_of_node):
    """Pack this core's edges (dst in [lo,hi)) into tiles of <=TILE_E edges and
    <=P distinct dsts, aligned to dst boundaries."""
    sel = (dst >= lo) & (dst < hi)
    s = src[sel]
    d = dst[sel]
    order = np.argsort(d, kind="stable")
    s = s[order]
    d = d[order]
    dl = (d - lo).astype(np.int64)

    nloc = hi - lo
    deg = np.bincount(dl, minlength=nloc)
    dst_start = np.concatenate([[0], np.cumsum(deg)])

    tiles = []
    i = 0
    while i < nloc:
        e0 = dst_start[i]
        j = i
        while j < nloc and (dst_start[j + 1] - e0) <= TILE_E and (j - i) < P:
            j += 1
        assert j > i, "single dst exceeds TILE_E edges"
        tiles.append((int(e0), int(dst_start[j]), i, j - i))
        i = j

    T = len(tiles)
    src_g = np.zeros((T, P, CHUNKS), np.int32)
    xr_l = np.zeros((T, P, CHUNKS), np.int32)
    dstloc = np.full((T, P, CHUNKS), 255.0, np.float32)
    outrow = np.full((T, P), np_pad, np.int32)

    for t, (e0, e1, d0, nd) in enumerate(tiles):
        ne = e1 - e0
        ss = s[e0:e1]
        gsrc = rank_of_node[ss].astype(np.int64) * np_pad + local_of_node[ss]
        flat_src = np.zeros(TILE_E, np.int32)
        flat_xr = np.zeros(TILE_E, np.int32)
        flat_dl = np.full(TILE_E, 255.0, np.float32)
        flat_src[:ne] = gsrc
        flat_xr[:ne] = dl[e0:e1]
        flat_dl[:ne] = dl[e0:e1] - d0
        src_g[t] = flat_src.reshape(CHUNKS, P).T
        xr_l[t] = flat_xr.reshape(CHUNKS, P).T
        dstloc[t] = flat_dl.reshape(CHUNKS, P).T
        outrow[t, :nd] = d0 + np.arange(nd)
    return src_g, xr_l, dstloc, outrow


def _aug_weights(W, b, att, C):
    """Columns: [1|h0|1|h1|1|h2|lin(3)].  ones come from the bias row; lin col h
    is 0.2*(W_h @ att_h) (leaky-relu linear part, pre-scaled)."""
    din = W.shape[0]
    Wout = 3 * (C + 1) + 3
    Wa = np.zeros((din, Wout), np.float32)
    ba = np.zeros((Wout,), np.float32)
    for h in range(HEADS):
        o = h * (C + 1)
        ba[o] = 1.0
        Wa[:, o + 1:o + 1 + C] = W[:, h * C:(h + 1) * C]
        ba[o + 1:o + 1 + C] = b[h * C:(h + 1) * C]
        Wa[:, 3 * (C + 1) + h] = NEG_SLOPE * (W[:, h * C:(h + 1) * C] @ att[h])
        ba[3 * (C + 1) + h] = NEG_SLOPE * float(b[h * C:(h + 1) * C] @ att[h])
    return Wa, ba


def _pad_rows(Wa, kpad):
    out = np.zeros((kpad, Wa.shape[1]), np.float32)
    out[:Wa.shape[0]] = Wa
    return out


def prepare(x, edge_index, batch_index, params):
    x = np.asarray(x, np.float32)
    ei = np.asarray(edge_index)
    bi = np.asarray(batch_index).astype(np.int64)
    n = x.shape[0]
    n_graphs = 256
    gpc = n_graphs // N_CORES

    loop = np.arange(n, dtype=np.int64)
    src = np.concatenate([ei[0].astype(np.int64), loop])
    dst = np.concatenate([ei[1].astype(np.int64), loop])

    bounds = np.searchsorted(bi, np.arange(0, n_graphs + 1, gpc))
    counts = np.diff(bounds)
    np_pad = (int(counts.max()) + 127) // 128 * 128
    NPP = np_pad + P

    rank_of_node = np.zeros(n, np.int32)
    local_of_node = np.zeros(n, np.int32)
    for k in range(N_CORES):
        lo, hi = int(bounds[k]), int(bounds[k + 1])
        rank_of_node[lo:hi] = k
        local_of_node[lo:hi] = np.arange(hi - lo, dtype=np.int32)

    per_core = [_build_edge_tiles(src, dst, int(bounds[k]), int(bounds[k + 1]),
                                  np_pad, rank_of_node, local_of_node)
                for k in range(N_CORES)]
    T = max(pc[0].shape[0] for pc in per_core)
    T = (T + UNROLL - 1) // UNROLL * UNROLL

    gsizes = np.bincount(bi, minlength=n_graphs)
    SMAX = 512
    while gsizes.max() >= SMAX:
        SMAX *= 2

    p = params
    layers = [
        dict(name="conv1", C=64, din=16, kpad=16, conv=p["conv1"], bn=p["bn1"], concat=True),
        dict(name="conv2", C=128, din=192, kpad=256, conv=p["conv2"], bn=p["bn2"], concat=True),
        dict(name="conv3", C=128, din=384, kpad=384, conv=p["conv3"], bn=p["bn3"], concat=True),
        dict(name="conv4", C=128, din=384, kpad=384, conv=p["conv4"], bn=p["bn4"], concat=True),
        dict(name="conv5", C=128, din=384, kpad=384, conv=p["conv5"], bn=p["bn5"], concat=False),
    ]

    W = {}
    for L in layers:
        C = L["C"]
        cv = L["conv"]
        att = np.asarray(cv["att"], np.float32)
        Wla, bla = _aug_weights(np.asarray(cv["Wl"], np.float32),
                                np.asarray(cv["bl"], np.float32), att, C)
        Wra, bra = _aug_weights(np.asarray(cv["Wr"], np.float32),
                                np.asarray(cv["br"], np.float32), att, C)
        nm = L["name"]
        W[nm + "_wl"] = _pad_rows(Wla, L["kpad"]).astype(BF16)
        W[nm + "_wr"] = _pad_rows(Wra, L["kpad"]).astype(BF16)
        W[nm + "_bl"] = bla[None, :].astype(BF16)
        W[nm + "_br"] = bra[None, :].astype(BF16)
        W[nm + "_att"] = np.ascontiguousarray(0.8 * att.T).astype(BF16)
        bn = L["bn"]
        gamma = np.asarray(bn["gamma"], np.float32)
        beta = np.asarray(bn["beta"], np.float32)
        mean = np.asarray(bn["mean"], np.float32)
        var = np.asarray(bn["var"], np.float32)
        extra = 1.0 if L["concat"] else (1.0 / 3.0)
        sc = gamma / np.sqrt(var + BN_EPS)
        sh = beta - mean * sc
        sc = sc * extra
        Cout = sc.shape[0]
        W[nm + "_s"] = np.ascontiguousarray(np.broadcast_to(sc, (P, Cout))).astype(np.float32)
        W[nm + "_t"] = np.ascontiguousarray(np.broadcast_to(sh, (P, Cout))).astype(np.float32)

    W["fc1w"] = np.asarray(p["fc1W"], np.float32).astype(BF16)
    W["fc2w"] = np.asarray(p["fc2W"], np.float32).astype(BF16)
    W["fc3w"] = np.asarray(p["fc3W"], np.float32).astype(BF16)
    W["fc1b"] = np.asarray(p["fc1b"], np.float32)[None, :].astype(BF16)
    W["fc2b"] = np.asarray(p["fc2b"], np.float32)[None, :].astype(BF16)
    W["fc3b"] = np.asarray(p["fc3b"], np.float32)[None, :].astype(BF16)
    W["iota"] = np.ascontiguousarray(
        np.broadcast_to(np.arange(P, dtype=np.float32), (P, P))).astype(BF16)
    W["ident"] = np.eye(P, dtype=np.float32).astype(BF16)
    tval = float(np.asarray(p["t"], np.float32))

    in_maps = []
    for k in range(N_CORES):
        lo, hi = int(bounds[k]), int(bounds[k + 1])
        nloc = hi - lo
        src_g, xr_l, dstloc, outrow = per_core[k]
        Tk = src_g.shape[0]

        srcmeta = np.zeros((T * P, 16), np.int32)
        dl_in = np.full((T * P, 8), 255.0, np.float32)
        srcmeta[:, 14] = np_pad           # default: trash row
        srcmeta[:, 15] = gpc * SMAX       # default: trash row of h5pad
        srcmeta[:Tk * P, 0:7] = src_g.reshape(Tk * P, CHUNKS)
        srcmeta[:Tk * P, 7:14] = xr_l.reshape(Tk * P, CHUNKS)
        srcmeta[:Tk * P, 14] = outrow.reshape(Tk * P)
        dl_in[:Tk * P, 0:7] = dstloc.reshape(Tk * P, CHUNKS)

        g_loc = (bi[lo:hi] - gpc * k).astype(np.int64)
        pos = np.zeros(nloc, np.int64)
        cnts = np.zeros(gpc, np.int64)
        for i_ in range(nloc):
            g = g_loc[i_]
            pos[i_] = cnts[g]
            cnts[g] += 1
        padrow = (g_loc * SMAX + pos).astype(np.int32)
        orr = srcmeta[:Tk * P, 14]
        valid = orr < np_pad
        srcmeta[:Tk * P, 15][valid] = padrow[orr[valid]]

        xT = np.zeros((16, np_pad), np.float32)
        xT[:, :nloc] = x[lo:hi].T
        gind = np.zeros((np_pad, 32), np.float32)
        gind[np.arange(nloc), g_loc] = 1.0
        cnt_inv = np.ascontiguousarray(np.broadcast_to(
            1.0 / np.maximum(cnts.astype(np.float32), 1.0), (P, gpc))).astype(np.float32)

        im = dict(W)
        im["xT1"] = xT.astype(BF16)
        im["srcmeta"] = srcmeta
        im["dstloc"] = dl_in
        im["gind"] = gind.astype(BF16)
        im["cntinv"] = cnt_inv
        im["tscal"] = np.full((P, 1), tval, np.float32)
        in_maps.append(im)

    meta = dict(np_pad=np_pad, NPP=NPP, T=T, SMAX=SMAX, gpc=gpc, layers=layers)
    return in_maps, meta


# ----------------------------------------------------------------------------
# Device kernel builder
# ----------------------------------------------------------------------------

H_WIDTHS = {"conv1": 256, "conv2": 384, "conv3": 384, "conv4": 384, "conv5": 128}


def build(nc, tc, meta):
    import os
    from contextlib import ExitStack
    import concourse.bass as bass
    import concourse.mybir as mybir
    from concourse.bass import IndirectOffsetOnAxis

    dt = mybir.dt
    AF = mybir.ActivationFunctionType
    OP = mybir.AluOpType
    ds = bass.ds

    np_pad = meta["np_pad"]
    NPP = meta["NPP"]
    T = meta["T"]
    SMAX = meta["SMAX"]
    gpc = meta["gpc"]
    layers = meta["layers"]
    NT = np_pad // P

    def ein(name, shape, dtype):
        return nc.dram_tensor(name, shape, dtype, kind="ExternalInput").ap()

    xT1 = ein("xT1", [16, np_pad], dt.bfloat16)
    srcmeta = ein("srcmeta", [T * P, 16], dt.int32)
    dstloc = ein("dstloc", [T * P, 8], dt.float32)
    gind_d = ein("gind", [np_pad, 32], dt.bfloat16)
    cntinv = ein("cntinv", [P, gpc], dt.float32)
    tscal = ein("tscal", [P, 1], dt.float32)
    wins = {}
    for L in layers:
        nm, C = L["name"], L["C"]
        Waug = 3 * (C + 1) + 3
        Cout = 3 * C if L["concat"] else C
        wins[nm + "_wl"] = ein(nm + "_wl", [L["kpad"], Waug], dt.bfloat16)
        wins[nm + "_wr"] = ein(nm + "_wr", [L["kpad"], Waug], dt.bfloat16)
        wins[nm + "_bl"] = ein(nm + "_bl", [1, Waug], dt.bfloat16)
        wins[nm + "_br"] = ein(nm + "_br", [1, Waug], dt.bfloat16)
        wins[nm + "_att"] = ein(nm + "_att", [C, 3], dt.bfloat16)
        wins[nm + "_s"] = ein(nm + "_s", [P, Cout], dt.float32)
        wins[nm + "_t"] = ein(nm + "_t", [P, Cout], dt.float32)
    for nm_, shape in [("fc1w", [512, 128]), ("fc2w", [128, 64]), ("fc3w", [64, 10]),
                       ("fc1b", [1, 128]), ("fc2b", [1, 64]), ("fc3b", [1, 10]),
                       ("iota", [P, P]), ("ident", [P, P])]:
        wins[nm_] = ein(nm_, shape, dt.bfloat16)

    out_d = nc.dram_tensor("out", [gpc, 10], dt.float32, kind="ExternalOutput").ap()

    def idram(name, shape, dtype, shared=False):
        return nc.dram_tensor(name, shape, dtype, kind="Internal",
                              addr_space=("Shared" if shared else "Local")).ap()

    h_bufs = {nm: idram("h_" + nm, [NPP, w], dt.bfloat16) for nm, w in H_WIDTHS.items()}
    h5pad = idram("h5pad", [gpc * SMAX + P, 128], dt.bfloat16)

    xl_loc, xr_loc, xl_full = {}, {}, {}
    for L in layers:
        nm = L["name"]
        Waug = 3 * (L["C"] + 1) + 3
        xl_loc[nm] = idram("xlloc_" + nm, [np_pad, Waug], dt.bfloat16)
        xr_loc[nm] = idram("xrloc_" + nm, [np_pad, Waug], dt.bfloat16)
        xl_full[nm] = idram("xlfull_" + nm, [N_CORES * np_pad, Waug], dt.bfloat16,
                            shared=True)

    rg = [list(range(N_CORES))]

    with ExitStack() as stack:
        cpool = stack.enter_context(tc.tile_pool(name="consts", bufs=1))
        iota_t = cpool.tile([P, P], dt.bfloat16, tag="iota")
        nc.sync.dma_start(iota_t[:], wins["iota"][:])
        ident = cpool.tile([P, P], dt.bfloat16, tag="ident")
        nc.sync.dma_start(ident[:], wins["ident"][:])
        ones1 = cpool.tile([1, P], dt.bfloat16, tag="ones1")
        nc.vector.memset(ones1[:], 1.0)
        ident3 = cpool.tile([HEADS, HEADS], dt.bfloat16, tag="ident3")
        nc.sync.dma_start(ident3[:], wins["ident"][0:HEADS, 0:HEADS])
        tsc_t = cpool.tile([P, 1], dt.float32, tag="tsc")
        nc.sync.dma_start(tsc_t[:], tscal[:])
        cti_t = cpool.tile([P, gpc], dt.float32, tag="cti")
        nc.sync.dma_start(cti_t[:], cntinv[:])

        wt = {}
        for L in layers:
            nm, C = L["name"], L["C"]
            Waug = 3 * (C + 1) + 3
            Cout = 3 * C if L["concat"] else C
            kp = L["kpad"]
            kb_ = min(kp, P)
            nfb_ = max(1, kp // P)
            for key in ("_wl", "_wr"):
                t_ = cpool.tile([kb_, nfb_, Waug], dt.bfloat16, tag=nm + key)
                for fb in range(nfb_):
                    nc.sync.dma_start(t_[:, fb, :],
                                      wins[nm + key][fb * P:fb * P + kb_, :])
                wt[nm + key] = t_
            for key, shape, dtp in [
                    ("_bl", [1, Waug], dt.bfloat16), ("_br", [1, Waug], dt.bfloat16),
                    ("_att", [C, 3], dt.bfloat16),
                    ("_s", [P, Cout], dt.float32), ("_t", [P, Cout], dt.float32)]:
                t_ = cpool.tile(shape, dtp, tag=nm + key)
                nc.sync.dma_start(t_[:], wins[nm + key][:])
                wt[nm + key] = t_
        t_ = cpool.tile([P, 4, 128], dt.bfloat16, tag="fc1w")
        for q in range(4):
            nc.sync.dma_start(t_[:, q, :], wins["fc1w"][q * P:(q + 1) * P, :])
        wt["fc1w"] = t_
        for nm_ in ("fc2w", "fc3w", "fc1b", "fc2b", "fc3b"):
            t_ = cpool.tile(list(wins[nm_].shape), dt.bfloat16, tag=nm_)
            nc.sync.dma_start(t_[:], wins[nm_][:])
            wt[nm_] = t_

        # zero h5pad and h_conv1 (padding columns/slots must be 0)
        zrow = cpool.tile([P, 1024], dt.bfloat16, tag="zrow")
        nc.vector.memset(zrow[:], 0.0)
        zn = P * 1024
        npad_rows = gpc * SMAX + P
        step = zn // 128
        for r0 in range(0, npad_rows, step):
            rr = min(step, npad_rows - r0)
            nc.sync.dma_start(h5pad[r0:r0 + rr, :], zrow[:, 0:rr * 128 // P])
        for hb, wdt in H_WIDTHS.items():
            step = zn // wdt
            for r0 in range(0, NPP, step):
                rr = min(step, NPP - r0)
                nc.sync.dma_start(h_bufs[hb][r0:r0 + rr, :],
                                  zrow[:, 0:rr * wdt // P])

        n_lay = int(os.environ.get("KLAYERS", "5"))
        do_ag = os.environ.get("KSKIP_AG", "") == ""
        do_edge = os.environ.get("KSKIP_EDGE", "") == ""
        do_ro = os.environ.get("KSKIP_RO", "") == ""
        prev_h = None
        for li, L in enumerate(layers[:n_lay]):
            nm, C = L["name"], L["C"]
            Waug = 3 * (C + 1) + 3
            Cout = 3 * C if L["concat"] else C
            kp = L["kpad"]
            nfb = max(1, kp // P)
            kb = P if li else 16  # contraction rows per block

            # ---- node phase ----
            with tc.tile_pool(name=nm + "np", bufs=3) as npool, \
                 tc.tile_pool(name=nm + "npp", bufs=2, space="PSUM") as nppool, \
                 tc.tile_pool(name=nm + "ht", bufs=1) as hpool:
                hT_blocks = []
                if li == 0:
                    hT = hpool.tile([16, np_pad], dt.bfloat16, tag="ht0")
                    nc.sync.dma_start(hT[:], xT1[:])
                    hT_blocks.append(hT)
                else:
                    for fb in range(nfb):
                        tb = hpool.tile([P, np_pad], dt.bfloat16, tag=f"ht{fb}")
                        nc.sync.dma_start(tb[:], prev_h[0:np_pad, fb * P:(fb + 1) * P],
                                          transpose=True)
                        hT_blocks.append(tb)

                def node_body(iv):
                    for si, (wkey, bkey, dest) in enumerate(
                            (("_wl", "_bl", xl_loc[nm]), ("_wr", "_br", xr_loc[nm]))):
                        ps = nppool.tile([P, Waug], dt.float32, tag="ps")
                        for fb in range(len(hT_blocks)):
                            lt = npool.tile([kb, P], dt.bfloat16, tag=f"lt{fb}{si}")
                            nc.sync.dma_start(lt[:], hT_blocks[fb][:, ds(iv * P, P)])
                            nc.tensor.matmul(ps[:], lhsT=lt[:],
                                             rhs=wt[nm + wkey][0:kb, fb, :],
                                             start=(fb == 0), stop=False)
                        nc.tensor.matmul(ps[:], lhsT=ones1[:], rhs=wt[nm + bkey][:],
                                         start=False, stop=True)
                        ot = npool.tile([P, Waug], dt.bfloat16, tag=f"ot{si}")
                        if si == 0:
                            nc.vector.tensor_copy(ot[:], ps[:])
                        else:
                            nc.scalar.copy(ot[:], ps[:])
                        nc.sync.dma_start(dest[ds(iv * P, P), :], ot[:])

                with tc.For_i(0, NT, 1) as iv:
                    node_body(iv)

            # ---- AllGather xl ----
            if do_ag:
                nc.gpsimd.collective_compute(
                    "AllGather", OP.bypass, replica_groups=rg,
                    ins=[xl_loc[nm][:]], outs=[xl_full[nm][:]])

            # ---- residual prefill (h4 = h2 + new) ----
            h_out = h_bufs[nm]
            if nm == "conv4":
                nc.sync.dma_start(h_out[:, :], h_bufs["conv2"][:, :])

            # ---- edge phase ----
            with tc.tile_pool(name=nm + "ep", bufs=3) as ep, \
                 tc.tile_pool(name=nm + "lh", bufs=2) as lhp, \
                 tc.tile_pool(name=nm + "pz", bufs=2, space="PSUM") as pz, \
                 tc.tile_pool(name=nm + "pp", bufs=1, space="PSUM") as pp:

                def edge_body(iv):
                    meta_t = ep.tile([P, 16], dt.int32, tag="meta")
                    nc.sync.dma_start(meta_t[:], srcmeta[ds(iv * P, P), :])
                    dl_t = ep.tile([P, 8], dt.float32, tag="dl")
                    nc.sync.dma_start(dl_t[:], dstloc[ds(iv * P, P), :])

                    zx = ep.tile([P, CHUNKS, Waug], dt.bfloat16, tag="zx")
                    xr = ep.tile([P, CHUNKS, Waug], dt.bfloat16, tag="xr")
                    for c in range(CHUNKS):
                        nc.gpsimd.indirect_dma_start(
                            out=zx[:, c, :], out_offset=None, in_=xl_full[nm][:],
                            in_offset=IndirectOffsetOnAxis(ap=meta_t[:, c:c + 1], axis=0))
                        nc.gpsimd.indirect_dma_start(
                            out=xr[:, c, :], out_offset=None, in_=xr_loc[nm][:],
                            in_offset=IndirectOffsetOnAxis(ap=meta_t[:, 7 + c:8 + c], axis=0))
                    z = ep.tile([P, CHUNKS, Waug], dt.bfloat16, tag="z")
                    nc.vector.tensor_tensor(out=z[:], in0=zx[:], in1=xr[:], op=OP.add)

                    # transpose head blocks into PSUM (bf16 PE transpose)
                    zT = [pz.tile([C, TILE_E], dt.bfloat16, tag=f"zT{h}", name=f"zT{h}")
                          for h in range(HEADS)]
                    for c in range(CHUNKS):
                        for h in range(HEADS):
                            nc.tensor.matmul(
                                zT[h][:, c * P:(c + 1) * P],
                                lhsT=z[:, c, h * (C + 1) + 1:(h + 1) * (C + 1)],
                                rhs=ident[:], start=True, stop=True, is_transpose=True)
                    zTr = [lhp.tile([C, TILE_E], dt.bfloat16, tag=f"zTr{h}", name=f"zTr{h}")
                           for h in range(HEADS)]
                    for h in range(HEADS):
                        nc.scalar.activation(zTr[h][:], zT[h][:], AF.Relu)

                    p_ps = pp.tile([P, CHUNKS, HEADS], dt.float32, tag="pps")
                    for c in range(CHUNKS):
                        for h in range(HEADS):
                            nc.tensor.matmul(
                                p_ps[:, c, h:h + 1],
                                lhsT=zTr[h][:, c * P:(c + 1) * P],
                                rhs=wt[nm + "_att"][:, h:h + 1],
                                start=True, stop=True)
                    logit = ep.tile([P, CHUNKS, HEADS], dt.float32, tag="logit")
                    nc.vector.tensor_tensor(
                        out=logit[:], in0=p_ps[:],
                        in1=z[:, :, 3 * (C + 1):3 * (C + 1) + 3], op=OP.add)
                    p_sb = ep.tile([P, CHUNKS, HEADS], dt.float32, tag="psb")
                    nc.scalar.activation(p_sb[:], logit[:], AF.Exp)

                    # p-scaled one-hot selection + aggregation matmuls
                    agg = pp.tile([P, HEADS, C + 1], dt.float32, tag="agg")
                    for h in range(HEADS):
                        for c in range(CHUNKS):
                            sel = lhp.tile([P, P], dt.bfloat16, tag="sel")
                            nc.vector.tensor_scalar(
                                out=sel[:], in0=iota_t[:],
                                scalar1=dl_t[:, c:c + 1], scalar2=p_sb[:, c, h:h + 1],
                                op0=OP.is_equal, op1=OP.mult)
                            nc.tensor.matmul(
                                agg[:, h, :], lhsT=sel[:],
                                rhs=zx[:, c, h * (C + 1):(h + 1) * (C + 1)],
                                start=(c == 0), stop=(c == CHUNKS - 1))

                    # epilogue: normalize, BN fold, relu, scatter
                    den_r = ep.tile([P, HEADS], dt.float32, tag="denr")
                    nc.vector.reciprocal(den_r[:], agg[:, :, 0])
                    if L["concat"]:
                        hn = ep.tile([P, Cout], dt.bfloat16, tag="hn")
                        for h in range(HEADS):
                            nc.vector.scalar_tensor_tensor(
                                out=hn[:, h * C:(h + 1) * C],
                                in0=agg[:, h, 1:C + 1], scalar=den_r[:, h:h + 1],
                                in1=wt[nm + "_s"][:, h * C:(h + 1) * C],
                                op0=OP.mult, op1=OP.mult)
                        hnf = ep.tile([P, Cout], dt.bfloat16, tag="hnf")
                        nc.vector.tensor_tensor(out=hnf[:], in0=hn[:],
                                                in1=wt[nm + "_t"][:], op=OP.add)
                        nc.vector.tensor_scalar_max(hnf[:], hnf[:], 0.0)
                    else:
                        acc = ep.tile([P, C], dt.float32, tag="acc5")
                        nc.vector.tensor_scalar(
                            out=acc[:], in0=agg[:, 0, 1:C + 1],
                            scalar1=den_r[:, 0:1], scalar2=None, op0=OP.mult)
                        for h in range(1, HEADS):
                            nc.vector.scalar_tensor_tensor(
                                out=acc[:], in0=agg[:, h, 1:C + 1],
                                scalar=den_r[:, h:h + 1], in1=acc[:],
                                op0=OP.mult, op1=OP.add)
                        hn = ep.tile([P, C], dt.bfloat16, tag="hn")
                        nc.vector.tensor_tensor(out=hn[:], in0=acc[:],
                                                in1=wt[nm + "_s"][:], op=OP.mult)
                        hnf = ep.tile([P, C], dt.bfloat16, tag="hnf")
                        nc.vector.tensor_tensor(out=hnf[:], in0=hn[:],
                                                in1=wt[nm + "_t"][:], op=OP.add)
                        nc.vector.tensor_scalar_max(hnf[:], hnf[:], 0.0)

                    cop = OP.add if nm == "conv4" else OP.bypass
                    nc.gpsimd.indirect_dma_start(
                        out=h_out[:],
                        out_offset=IndirectOffsetOnAxis(ap=meta_t[:, 14:15], axis=0),
                        in_=hnf[:], in_offset=None, compute_op=cop)
                    if nm == "conv5":
                        nc.gpsimd.indirect_dma_start(
                            out=h5pad[:],
                            out_offset=IndirectOffsetOnAxis(ap=meta_t[:, 15:16], axis=0),
                            in_=hnf[:], in_offset=None)

                if do_edge:
                    with tc.For_i(0, T, UNROLL) as iv:
                        for u in range(UNROLL):
                            edge_body(iv + u)

            prev_h = h_bufs[nm]

        # ------------------------------------------------------------------
        # readout
        # ------------------------------------------------------------------
        if not do_ro:
            dummy = stack.enter_context(tc.tile_pool(name="dummy", bufs=1))
            dz = dummy.tile([gpc, 10], dt.float32, tag="dz")
            nc.vector.memset(dz[:], 0.0)
            nc.sync.dma_start(out_d[:], dz[:])
            return
        h5 = h_bufs["conv5"]
        rbig = stack.enter_context(tc.tile_pool(name="rbig", bufs=1))
        sum_s = rbig.tile([P, gpc], dt.float32, tag="sum_s")
        sq_s = rbig.tile([P, gpc], dt.float32, tag="sq_s")
        den_s = rbig.tile([P, gpc], dt.float32, tag="den_s")
        num_s = rbig.tile([P, gpc], dt.float32, tag="num_s")

        with tc.tile_pool(name="ro", bufs=3) as ro, \
             tc.tile_pool(name="rop", bufs=1, space="PSUM") as rop:
            sumT = rop.tile([P, gpc], dt.float32, tag="sumT")
            sqT = rop.tile([P, gpc], dt.float32, tag="sqT")
            denT = rop.tile([P, gpc], dt.float32, tag="denT")
            numT = rop.tile([P, gpc], dt.float32, tag="numT")
            for t_ in (sumT, sqT, denT, numT):
                nc.vector.memset(t_[:], 0.0)

            def ro_body(iv):
                ht = ro.tile([P, P], dt.bfloat16, tag="ht")
                nc.sync.dma_start(ht[:], h5[ds(iv * P, P), 0:P])
                gi = ro.tile([P, gpc], dt.bfloat16, tag="gi")
                nc.sync.dma_start(gi[:], gind_d[ds(iv * P, P), :])
                sq = ro.tile([P, P], dt.bfloat16, tag="sq")
                nc.vector.tensor_tensor(out=sq[:], in0=ht[:], in1=ht[:], op=OP.mult)
                ex = ro.tile([P, P], dt.bfloat16, tag="ex")
                nc.scalar.activation(ex[:], ht[:], AF.Exp, scale=tsc_t[:])
                exh = ro.tile([P, P], dt.bfloat16, tag="exh")
                nc.vector.tensor_tensor(out=exh[:], in0=ex[:], in1=ht[:], op=OP.mult)
                for psum_t, lhs in ((sumT, ht), (sqT, sq), (denT, ex), (numT, exh)):
                    nc.tensor.matmul(psum_t[:], lhsT=lhs[:], rhs=gi[:],
                                     start=False, stop=False, skip_group_check=True)

            with tc.For_i(0, NT, 1) as iv:
                ro_body(iv)

            for psum_t, sb in ((sumT, sum_s), (sqT, sq_s), (denT, den_s), (numT, num_s)):
                nc.vector.tensor_copy(sb[:], psum_t[:])

        with tc.tile_pool(name="ro2", bufs=1) as ro2, \
             tc.tile_pool(name="rop2", bufs=1, space="PSUM") as rop2:
            # segment max via graph-padded transpose
            h5pT = ro2.tile([P, gpc * SMAX], dt.bfloat16, tag="h5pT")
            nc.sync.dma_start(h5pT[:], h5pad[0:gpc * SMAX, :], transpose=True)
            mxT = ro2.tile([P, gpc], dt.float32, tag="mxT")
            nc.vector.tensor_reduce(
                out=mxT[:], in_=h5pT[:].rearrange("p (g s) -> p g s", g=gpc),
                axis=mybir.AxisListType.X, op=OP.max)

            meanT = ro2.tile([P, gpc], dt.float32, tag="meanT")
            nc.vector.tensor_tensor(out=meanT[:], in0=sum_s[:], in1=cti_t[:], op=OP.mult)
            m2T = ro2.tile([P, gpc], dt.float32, tag="m2T")
            nc.vector.tensor_tensor(out=m2T[:], in0=sq_s[:], in1=cti_t[:], op=OP.mult)
            mm = ro2.tile([P, gpc], dt.float32, tag="mm")
            nc.vector.tensor_tensor(out=mm[:], in0=meanT[:], in1=meanT[:], op=OP.mult)
            var = ro2.tile([P, gpc], dt.float32, tag="var")
            nc.vector.tensor_tensor(out=var[:], in0=m2T[:], in1=mm[:], op=OP.subtract)
            nc.vector.tensor_scalar_max(var[:], var[:], 0.0)
            nc.vector.tensor_scalar_add(var[:], var[:], 1e-5)
            stdT = ro2.tile([P, gpc], dt.float32, tag="stdT")
            nc.scalar.activation(stdT[:], var[:], AF.Sqrt)
            dinv = ro2.tile([P, gpc], dt.float32, tag="dinv")
            nc.vector.reciprocal(dinv[:], den_s[:])
            smT = ro2.tile([P, gpc], dt.float32, tag="smT")
            nc.vector.tensor_tensor(out=smT[:], in0=num_s[:], in1=dinv[:], op=OP.mult)

            rT = []
            for i_, src_t in enumerate((meanT, stdT, mxT, smT)):
                tb = ro2.tile([P, gpc], dt.bfloat16, tag=f"rT{i_}")
                nc.vector.tensor_copy(tb[:], src_t[:])
                rT.append(tb)

            o1 = rop2.tile([gpc, 128], dt.float32, tag="o1")
            for q in range(4):
                nc.tensor.matmul(o1[:], lhsT=rT[q][:], rhs=wt["fc1w"][:, q, :],
                                 start=(q == 0), stop=False)
            nc.tensor.matmul(o1[:], lhsT=ones1[:, 0:gpc], rhs=wt["fc1b"][:],
                             start=False, stop=True)
            o1s = ro2.tile([gpc, 128], dt.bfloat16, tag="o1s")
            nc.vector.tensor_copy(o1s[:], o1[:])
            o1T = rop2.tile([P, gpc], dt.float32, tag="o1T")
            nc.tensor.matmul(o1T[:], lhsT=o1s[:], rhs=ident[0:gpc, 0:gpc],
                             start=True, stop=True)
            o1Ts = ro2.tile([P, gpc], dt.bfloat16, tag="o1Ts")
            nc.vector.tensor_copy(o1Ts[:], o1T[:])

            o2 = rop2.tile([gpc, 64], dt.float32, tag="o2")
            nc.tensor.matmul(o2[:], lhsT=o1Ts[:], rhs=wt["fc2w"][:], start=True, stop=False)
            nc.tensor.matmul(o2[:], lhsT=ones1[:, 0:gpc], rhs=wt["fc2b"][:],
                             start=False, stop=True)
            o2s = ro2.tile([gpc, 64], dt.bfloat16, tag="o2s")
            nc.vector.tensor_copy(o2s[:], o2[:])
            o2T = rop2.tile([64, gpc], dt.float32, tag="o2T")
            nc.tensor.matmul(o2T[:], lhsT=o2s[:], rhs=ident[0:gpc, 0:gpc],
                             start=True, stop=True)
            o2Ts = ro2.tile([64, gpc], dt.bfloat16, tag="o2Ts")
            nc.vector.tensor_copy(o2Ts[:], o2T[:])

            o3 = rop2.tile([gpc, 10], dt.float32, tag="o3")
            nc.tensor.matmul(o3[:], lhsT=o2Ts[:], rhs=wt["fc3w"][:], start=True, stop=False)
            nc.tensor.matmul(o3[:], lhsT=ones1[:, 0:gpc], rhs=wt["fc3b"][:],
                             start=False, stop=True)
            o3s = ro2.tile([gpc, 10], dt.float32, tag="o3s")
            nc.vector.tensor_copy(o3s[:], o3[:])
            nc.sync.dma_start(out_d[:], o3s[:])


def build_module(meta):
    import concourse.bacc as bacc
    import concourse.tile as tile

    nc = bacc.Bacc("TRN2", target_bir_lowering=False, debug=False,
                   enable_asserts=False, num_devices=N_CORES)
    with tile.TileContext(nc) as tc:
        build(nc, tc, meta)
    nc.compile()
    return nc


def kernel(x, edge_index, batch_index, params):
    import concourse.bass_utils as bass_utils

    in_maps, meta = prepare(x, edge_index, batch_index, params)
    nc = build_module(meta)
    res = bass_utils.run_bass_kernel_spmd(nc, in_maps, core_ids=list(range(N_CORES)))
    outs = [res.results[k]["out"] for k in range(N_CORES)]
    return np.concatenate(outs, axis=0).astype(np.float32)
